# revision 16
# baseline (speedup 1.0000x reference)
"""GCN + 4x GAT encoder on 8 Trainium2 NeuronCores (Bass/Tile).

Sharding: destination-node sharding (2500 nodes/core). Host sorts edges by
destination and pads each (core, dest-block) edge list to a uniform tile
count so one SPMD program serves all cores. Per layer:
  dense:  xw/als/ald for the local node shard (PE), packed row table ->
          local DRAM, AllGather -> full table in every core's DRAM.
  sparse: dma_gather source rows per edge (prepare_only + trigger so the
          GpSimd engine only pays descriptor-gen), gather dest rows (ald),
          exp(leaky_relu(z)) = max(exp(z), exp(0.2 z)); weighted features
          use an interleaved row layout [xw_h(64) | 1] * 4 heads so the
          per-edge exp lands inline as the softmax-denominator column; a
          one-hot matmul per 128-edge tile segment-sums into the dest
          block's PSUM (edges sorted by dest => each tile hits one 128-dest
          window). alpha = exp * recip(segment sum), gathered per edge.
"""

import numpy as np

import concourse.bass as bass
import concourse.tile as tile
from concourse import bacc, mybir
from concourse.bass_utils import run_bass_kernel_spmd

N = 20000
E = 320000
EP = E + N           # with self loops
NCORES = 8
VSH = N // NCORES    # 2500 dest nodes per core
NBLK = (VSH + 127) // 128          # 20 dest blocks (last = 68 rows)
G = 8                # tiles per gather group (G*128 edges per dma_gather)
F32 = mybir.dt.float32
I16 = mybir.dt.int16

NEG_SLOPE = 0.2
EPSV = 1e-16


def _interleave_cols(Wm):
    """[Cin, 256] -> [Cin, 260] with col 65h+c = W[:, 64h+c], col 65h+64 = 0."""
    cin = Wm.shape[0]
    out = np.zeros((cin, 260), np.float32)
    for h in range(4):
        out[:, 65 * h:65 * h + 64] = Wm[:, 64 * h:64 * (h + 1)]
    return out


def _blockdiag(a):
    h, c = a.shape
    out = np.zeros((h * c, h), np.float32)
    for i in range(h):
        out[i * c:(i + 1) * c, i] = a[i]
    return out


def _wrap16(idx):
    w = idx.reshape(-1, 16).T.astype(np.int16)
    return np.ascontiguousarray(np.tile(w, (8, 1)))


def _prep(edge_index):
    s = np.concatenate([edge_index[0], np.arange(N, dtype=np.int64)])
    d = np.concatenate([edge_index[1], np.arange(N, dtype=np.int64)])
    deg = np.bincount(d, minlength=N).astype(np.float32)
    dis = np.where(deg > 0, 1.0 / np.sqrt(deg), 0.0).astype(np.float32)
    normv = (dis[s] * dis[d]).astype(np.float32)

    perm = np.argsort(d, kind="stable")
    ss, dd, nn = s[perm], d[perm], normv[perm]

    core = dd // VSH
    blk = (dd % VSH) // 128
    cnt = np.zeros((NCORES, NBLK), np.int64)
    np.add.at(cnt, (core, blk), 1)
    tpb = np.maximum(1, (cnt.max(axis=0) + 127) // 128)
    tile_off = np.concatenate([[0], np.cumsum(tpb)])
    T = int(tile_off[-1])
    EPAD = T * 128

    per_core = []
    for k in range(NCORES):
        sidx = np.zeros(EPAD, np.int64)
        dloc = np.zeros(EPAD, np.int64)
        dblk = np.full(EPAD, -1.0, np.float32)
        nrm = np.zeros(EPAD, np.float32)
        spos = np.full(EPAD, -1, np.int64)
        for b in range(NBLK):
            m = (core == k) & (blk == b)
            idxs = np.nonzero(m)[0]
            c = len(idxs)
            o = int(tile_off[b]) * 128
            sidx[o:o + c] = ss[idxs]
            dloc[o:o + c] = dd[idxs] - k * VSH
            dblk[o:o + c] = (dd[idxs] - k * VSH - b * 128).astype(np.float32)
            nrm[o:o + c] = nn[idxs]
            spos[o:o + c] = idxs
        per_core.append(dict(
            sidx_w=_wrap16(sidx),
            dloc_w=_wrap16(dloc),
            dblk_part=np.ascontiguousarray(dblk.reshape(T, 128).T)[:, :, None],
            norm_part=np.ascontiguousarray(nrm.reshape(T, 128).T)[:, :, None],
            spos=spos,
        ))
    return per_core, tpb.tolist(), tile_off, T, perm


# ---------------------------------------------------------------- program ---

def _build(tpb, T):
    nc = bacc.Bacc("TRN2", target_bir_lowering=False, debug=False,
                   num_devices=NCORES, num_swdge_queues=4)
    EPAD = T * 128
    NGRP = (T + G - 1) // G

    def din(name, shape, dtyp=F32):
        return nc.dram_tensor(name, list(shape), dtyp, kind="ExternalInput")

    xT = din("xT", [128, VSH])
    sidx_in = din("sidx", [128, EPAD // 16], I16)
    dloc_in = din("dloc", [128, EPAD // 16], I16)
    dblk_in = din("dblk", [128, T, 1])
    norm_in = din("norm", [128, T, 1])
    iota_in = din("iota", [128, 1, 128])
    ident_in = din("ident", [128, 128])
    rhs0_in = din("rhs0", [128, 64])
    rhs1_in = din("rhs1", [64, 268])
    rhs2a_in = din("rhs2a", [128, 268])
    rhs2b_in = din("rhs2b", [128, 268])
    rhsm_in = din("rhsm", [64, 268])
    rhss_in = din("rhss", [64, 268])
    b0_in = din("b0b", [128, 64])
    b1_in = din("b1b", [128, 256])
    b2_in = din("b2b", [128, 64])
    bm_in = din("bmb", [128, 64])
    bs_in = din("bsb", [128, 64])

    zm_out = nc.dram_tensor("zm", [VSH, 64], F32, kind="ExternalOutput")
    zs_out = nc.dram_tensor("zs", [VSH, 64], F32, kind="ExternalOutput")
    a_outs = {nm: nc.dram_tensor(nm, [128, T, 4], F32, kind="ExternalOutput")
              for nm in ("a1", "a2", "am", "as_")}

    def internal(name, shape, shared=False):
        return nc.dram_tensor(name, list(shape), F32, kind="Internal",
                              addr_space="Shared" if shared else "Local")

    # GAT row: [interleaved xw|1 (260) | als 4] pad-> 320
    # ms row:  [ilv_m 260 | ilv_s 260 | als_m 4 | als_s 4] pad-> 576
    T0_loc = internal("T0_loc", [VSH, 64]);   T0 = internal("T0", [N, 64], True)
    T1_loc = internal("T1_loc", [VSH, 320]);  T1 = internal("T1", [N, 320], True)
    T2_loc = internal("T2_loc", [VSH, 320]);  T2 = internal("T2", [N, 320], True)
    Tms_loc = internal("Tms_loc", [VSH, 576]); Tms = internal("Tms", [N, 576], True)
    D1 = internal("D1", [VSH, 64]); D2 = internal("D2", [VSH, 64])
    Dms = internal("Dms", [VSH, 64])
    R1 = internal("R1", [VSH, 64]); R2 = internal("R2", [VSH, 64])
    Rms = internal("Rms", [VSH, 64])

    tile_off = np.concatenate([[0], np.cumsum(tpb)]).astype(int)
    blk_of = np.zeros(T, int)
    for b in range(NBLK):
        blk_of[tile_off[b]:tile_off[b + 1]] = b
    first_of = {int(tile_off[b]): b for b in range(NBLK)}
    last_of = {int(tile_off[b + 1] - 1): b for b in range(NBLK)}
    nrows_blk = [min(128, VSH - b * 128) for b in range(NBLK)]

    dma_sems = [nc.alloc_semaphore(f"swdge_dma_q{q}") for q in range(4)]

    from contextlib import ExitStack
    with tile.TileContext(nc) as tc, ExitStack() as _es:
        cpool = _es.enter_context(tc.tile_pool(name="consts", bufs=1))
        gpool = _es.enter_context(tc.tile_pool(name="gather", bufs=3))
        wpool = _es.enter_context(tc.tile_pool(name="wf", bufs=4))
        opool = _es.enter_context(tc.tile_pool(name="onehot", bufs=3))
        ppool = _es.enter_context(tc.tile_pool(name="psum", bufs=2, space="PSUM"))
        dpool = _es.enter_context(tc.tile_pool(name="dpsum", bufs=2, space="PSUM"))
        tpool = _es.enter_context(tc.tile_pool(name="tpsum", bufs=2, space="PSUM"))
        fpool = _es.enter_context(tc.tile_pool(name="flush", bufs=6))
        hpool = _es.enter_context(tc.tile_pool(name="hT", bufs=3))
        epool = _es.enter_context(tc.tile_pool(name="expst", bufs=3))
        rpool = _es.enter_context(tc.tile_pool(name="rows", bufs=2))

        def load(nm, ap_in, shape, dtyp=F32, pool=cpool):
            t = pool.tile(list(shape), dtyp, name=nm, tag=nm)
            nc.sync.dma_start(t[:], ap_in[:])
            return t

        sidx_t = load("sidxT", sidx_in, [128, EPAD // 16], I16)
        dloc_t = load("dlocT", dloc_in, [128, EPAD // 16], I16)
        dblk_t = load("dblkT", dblk_in, [128, T, 1])
        norm_t = load("normT", norm_in, [128, T, 1])
        iota_t = load("iotaT", iota_in, [128, 1, 128])
        ident_t = load("identT", ident_in, [128, 128])
        rhs0_t = load("rhs0T", rhs0_in, [128, 64])
        rhs1_t = load("rhs1T", rhs1_in, [64, 268])
        rhs2a_t = load("rhs2aT", rhs2a_in, [128, 268])
        rhs2b_t = load("rhs2bT", rhs2b_in, [128, 268])
        rhsm_t = load("rhsmT", rhsm_in, [64, 268])
        rhss_t = load("rhssT", rhss_in, [64, 268])
        b0_t = load("b0T", b0_in, [128, 64])
        b1_t = load("b1T", b1_in, [128, 256])
        b2_t = load("b2T", b2_in, [128, 64])
        bm_t = load("bmT", bm_in, [128, 64])
        bs_t = load("bsT", bs_in, [128, 64])
        xT_t = load("xTT", xT, [128, VSH])

        AF = mybir.ActivationFunctionType

        def gather(dst_ap, src_ap, idx_ap, nidx, rowf, q):
            nc.gpsimd.dma_gather(dst_ap, src_ap, idx_ap, nidx, nidx, rowf,
                                 prepare_only=True, sem=dma_sems[q],
                                 queue_num=q)
            nc.gpsimd.trigger_dma(count=None, queue_num=q)

        def build_onehot(t0, gn):
            oh = opool.tile([128, G, 128], F32, tag="oh", name="oh")
            nc.vector.tensor_tensor(
                out=oh[:, :gn, :],
                in0=dblk_t[:, t0:t0 + gn, :].to_broadcast([128, gn, 128]),
                in1=iota_t[:, :, :].to_broadcast([128, gn, 128]),
                op=mybir.AluOpType.is_equal)
            return oh

        def dense_phase(hT_tiles, rhs_list, tloc, rowf, dloc_dram):
            for t in range(NBLK):
                nt = nrows_blk[t]
                o = t * 128
                ps = dpool.tile([128, 268], F32, tag="dps", name="dps")
                nchunk = len(hT_tiles)
                for kc in range(nchunk):
                    hT_t, kk = hT_tiles[kc]
                    nc.tensor.matmul(
                        out=ps[:nt, :], lhsT=hT_t[:kk, o:o + nt],
                        rhs=rhs_list[kc][:kk, :],
                        start=(kc == 0), stop=(kc == nchunk - 1))
                row = rpool.tile([128, rowf], F32, tag="row", name="rowt")
                nc.scalar.copy(row[:nt, :264], ps[:nt, :264])
                for h in range(4):
                    nc.vector.memset(row[:nt, 65 * h + 64:65 * h + 65], 1.0)
                nc.sync.dma_start(tloc[o:o + nt, :], row[:nt, :])
                drow = rpool.tile([128, 64], F32, tag="drow", name="drowt")
                nc.scalar.copy(drow[:nt, :4], ps[:nt, 264:268])
                nc.sync.dma_start(dloc_dram[o:o + nt, :], drow[:nt, :])

        def dense_phase_ms():
            for t in range(NBLK):
                nt = nrows_blk[t]
                o = t * 128
                psm = dpool.tile([128, 268], F32, tag="dps", name="dpsm")
                pss = dpool.tile([128, 268], F32, tag="dps", name="dpss")
                nc.tensor.matmul(out=psm[:nt, :], lhsT=h2T_t[:64, o:o + nt],
                                 rhs=rhsm_t[:64, :], start=True, stop=True)
                nc.tensor.matmul(out=pss[:nt, :], lhsT=h2T_t[:64, o:o + nt],
                                 rhs=rhss_t[:64, :], start=True, stop=True)
                row = rpool.tile([128, 576], F32, tag="row", name="rowms")
                nc.scalar.copy(row[:nt, :260], psm[:nt, :260])
                nc.scalar.copy(row[:nt, 260:520], pss[:nt, :260])
                nc.scalar.copy(row[:nt, 520:524], psm[:nt, 260:264])
                nc.scalar.copy(row[:nt, 524:528], pss[:nt, 260:264])
                for h in range(4):
                    nc.vector.memset(row[:nt, 65 * h + 64:65 * h + 65], 1.0)
                    nc.vector.memset(row[:nt, 260 + 65 * h + 64:
                                         260 + 65 * h + 65], 1.0)
                nc.sync.dma_start(Tms_loc[o:o + nt, :], row[:nt, :])
                drow = rpool.tile([128, 64], F32, tag="drow", name="drowt")
                nc.scalar.copy(drow[:nt, :4], psm[:nt, 264:268])
                nc.scalar.copy(drow[:nt, 4:8], pss[:nt, 264:268])
                nc.sync.dma_start(Dms[o:o + nt, :], drow[:nt, :])

        def allgather(src, dst):
            nc.gpsimd.collective_compute(
                "AllGather", mybir.AluOpType.bypass,
                replica_groups=[list(range(NCORES))],
                ins=[src.ap().opt()], outs=[dst.ap().opt()])

        def exp_lrelu(dst_ap, lg_ap, gn):
            """dst = exp(leaky_relu(lg)) = max(exp(lg), exp(0.2*lg))."""
            e2 = wpool.tile([128, G, 4], F32, tag="lg", name="e2")
            nc.scalar.activation(e2[:, :gn, :], lg_ap, AF.Exp, scale=NEG_SLOPE)
            nc.scalar.activation(dst_ap, lg_ap, AF.Exp)
            nc.vector.tensor_tensor(out=dst_ap, in0=dst_ap, in1=e2[:, :gn, :],
                                    op=mybir.AluOpType.max)

        def sparse_gcn():
            hT_new = hpool.tile([128, VSH], F32, tag="hT", name="h0T")
            psums = {}
            for g in range(NGRP):
                t0 = g * G
                gn = min(G, T - t0)
                src = gpool.tile([128, G, 64], F32, tag="src", name="src")
                gather(src[:, :gn, :], T0.ap(), sidx_t[:, t0 * 8:(t0 + gn) * 8],
                       gn * 128, 64, g % 3)
                wf = wpool.tile([128, G, 64], F32, tag="wf", name="wf")
                nc.vector.tensor_mul(
                    wf[:, :gn, :], src[:, :gn, :],
                    norm_t[:, t0:t0 + gn, :].to_broadcast([128, gn, 64]))
                oh = build_onehot(t0, gn)
                for ti in range(gn):
                    t = t0 + ti
                    b = blk_of[t]
                    if t in first_of:
                        psums[b] = ppool.tile([128, 64], F32, tag="agg",
                                              name="agg0")
                    nc.tensor.matmul(out=psums[b][:], lhsT=oh[:, ti, :],
                                     rhs=wf[:, ti, :],
                                     start=(t == tile_off[b]),
                                     stop=(t == tile_off[b + 1] - 1))
                    if t in last_of:
                        b = last_of[t]
                        nb = nrows_blk[b]
                        hsb = fpool.tile([128, 64], F32, tag="hf", name="h0f")
                        nc.vector.tensor_add(hsb[:nb, :], psums[b][:nb, :],
                                             b0_t[:nb, :])
                        nc.scalar.activation(hsb[:nb, :], hsb[:nb, :], AF.Relu)
                        tp = tpool.tile([128, 128], F32, tag="tp", name="tp")
                        nc.tensor.transpose(out=tp[:64, :nb], in_=hsb[:nb, :64],
                                            identity=ident_t[:nb, :nb])
                        nc.scalar.copy(hT_new[:64, b * 128:b * 128 + nb],
                                       tp[:64, :nb])
                        del psums[b]
            return hT_new

        def flush_concat(ps, nb, rec, bias_t):
            hsb = fpool.tile([128, 256], F32, tag="hf", name="h1f")
            for h in range(4):
                nc.vector.tensor_scalar_mul(
                    hsb[:nb, h * 64:(h + 1) * 64],
                    ps[:nb, 65 * h:65 * h + 64], rec[:nb, h:h + 1])
            nc.vector.tensor_add(hsb[:nb, :], hsb[:nb, :], bias_t[:nb, :])
            nc.scalar.activation(hsb[:nb, :], hsb[:nb, :], AF.Relu)
            return hsb

        def flush_mean(ps, nb, rec, bias_t):
            rec4 = fpool.tile([128, 4], F32, tag="rec", name="rec4")
            nc.vector.tensor_scalar_mul(rec4[:nb, :], rec[:nb, :], 0.25)
            acc = fpool.tile([128, 64], F32, tag="hf", name="hmf")
            nc.vector.tensor_scalar_mul(acc[:nb, :], ps[:nb, 0:64],
                                        rec4[:nb, 0:1])
            for h in range(1, 4):
                tmp = fpool.tile([128, 64], F32, tag="hf", name="hmt")
                nc.vector.tensor_scalar_mul(
                    tmp[:nb, :], ps[:nb, 65 * h:65 * h + 64],
                    rec4[:nb, h:h + 1])
                nc.vector.tensor_add(acc[:nb, :], acc[:nb, :], tmp[:nb, :])
            nc.vector.tensor_add(acc[:nb, :], acc[:nb, :], bias_t[:nb, :])
            return acc

        def make_rec(ps, nb):
            rec = fpool.tile([128, 4], F32, tag="rec", name="rec")
            for h in range(4):
                nc.scalar.copy(rec[:nb, h:h + 1],
                               ps[:nb, 65 * h + 64:65 * h + 65])
            nc.vector.tensor_scalar_add(rec[:nb, :], rec[:nb, :], EPSV)
            nc.vector.reciprocal(rec[:nb, :], rec[:nb, :])
            return rec

        def sparse_gat(Tfull, Dl, Rl, rowf, exp_store, bias_t, concat,
                       out_hT_parts, alpha_out, z_dram=None):
            if out_hT_parts:
                hT_new = [hpool.tile([128, VSH], F32, tag="hT",
                                     name=f"hTn{i}")
                          for i in range(out_hT_parts)]
            else:
                hT_new = None
            psums = {}
            for g in range(NGRP):
                t0 = g * G
                gn = min(G, T - t0)
                src = gpool.tile([128, G, rowf], F32, tag="src", name="src")
                gather(src[:, :gn, :], Tfull.ap(),
                       sidx_t[:, t0 * 8:(t0 + gn) * 8], gn * 128, rowf, g % 3)
                dstr = gpool.tile([128, G, 64], F32, tag="dst", name="dst")
                gather(dstr[:, :gn, :], Dl.ap(),
                       dloc_t[:, t0 * 8:(t0 + gn) * 8], gn * 128, 64, 3)
                lg = wpool.tile([128, G, 4], F32, tag="lg", name="lg")
                nc.vector.tensor_add(lg[:, :gn, :], src[:, :gn, 260:264],
                                     dstr[:, :gn, 0:4])
                exp_lrelu(exp_store[:, t0:t0 + gn, :], lg[:, :gn, :], gn)
                wf = wpool.tile([128, G, 260], F32, tag="wf", name="wf")
                for h in range(4):
                    nc.vector.tensor_mul(
                        wf[:, :gn, 65 * h:65 * h + 65],
                        src[:, :gn, 65 * h:65 * h + 65],
                        exp_store[:, t0:t0 + gn, h:h + 1].to_broadcast(
                            [128, gn, 65]))
                oh = build_onehot(t0, gn)
                for ti in range(gn):
                    t = t0 + ti
                    b = blk_of[t]
                    if t in first_of:
                        psums[b] = ppool.tile([128, 260], F32, tag="agg",
                                              name="aggg")
                    nc.tensor.matmul(out=psums[b][:], lhsT=oh[:, ti, :],
                                     rhs=wf[:, ti, :],
                                     start=(t == tile_off[b]),
                                     stop=(t == tile_off[b + 1] - 1))
                    if t in last_of:
                        b = last_of[t]
                        nb = nrows_blk[b]
                        ps = psums[b]
                        rec = make_rec(ps, nb)
                        rrow = rpool.tile([128, 64], F32, tag="rrow",
                                          name="rrow")
                        nc.scalar.copy(rrow[:nb, :4], rec[:nb, :4])
                        nc.sync.dma_start(Rl[b * 128:b * 128 + nb, :],
                                          rrow[:nb, :])
                        if concat:
                            hsb = flush_concat(ps, nb, rec, bias_t)
                            for kc in range(2):
                                tp = tpool.tile([128, 128], F32, tag="tp",
                                                name="tp")
                                nc.tensor.transpose(
                                    out=tp[:, :nb],
                                    in_=hsb[:nb, kc * 128:(kc + 1) * 128],
                                    identity=ident_t[:nb, :nb])
                                nc.scalar.copy(
                                    hT_new[kc][:, b * 128:b * 128 + nb],
                                    tp[:, :nb])
                        else:
                            acc = flush_mean(ps, nb, rec, bias_t)
                            if z_dram is not None:
                                nc.sync.dma_start(
                                    z_dram[b * 128:b * 128 + nb, :],
                                    acc[:nb, :])
                            else:
                                nc.scalar.activation(acc[:nb, :], acc[:nb, :],
                                                     AF.Relu)
                                tp = tpool.tile([128, 128], F32, tag="tp",
                                                name="tp")
                                nc.tensor.transpose(out=tp[:64, :nb],
                                                    in_=acc[:nb, :64],
                                                    identity=ident_t[:nb, :nb])
                                nc.scalar.copy(
                                    hT_new[0][:64, b * 128:b * 128 + nb],
                                    tp[:64, :nb])
                        del psums[b]
            # alpha pass
            for g in range(NGRP):
                t0 = g * G
                gn = min(G, T - t0)
                rt = gpool.tile([128, G, 64], F32, tag="dst", name="rt")
                gather(rt[:, :gn, :], Rl.ap(), dloc_t[:, t0 * 8:(t0 + gn) * 8],
                       gn * 128, 64, 3)
                at = wpool.tile([128, G, 4], F32, tag="at", name="at")
                nc.vector.tensor_mul(at[:, :gn, :], exp_store[:, t0:t0 + gn, :],
                                     rt[:, :gn, 0:4])
                nc.sync.dma_start(alpha_out[:, t0:t0 + gn, :], at[:, :gn, :])
            return hT_new

        def sparse_ms():
            psums_m, psums_s = {}, {}
            for g in range(NGRP):
                t0 = g * G
                gn = min(G, T - t0)
                src = gpool.tile([128, G, 576], F32, tag="src", name="src")
                gather(src[:, :gn, :], Tms.ap(),
                       sidx_t[:, t0 * 8:(t0 + gn) * 8], gn * 128, 576, g % 3)
                dstr = gpool.tile([128, G, 64], F32, tag="dst", name="dst")
                gather(dstr[:, :gn, :], Dms.ap(),
                       dloc_t[:, t0 * 8:(t0 + gn) * 8], gn * 128, 64, 3)
                for (nm, alo, dlo, es) in (("m", 520, 0, expm_t),
                                           ("s", 524, 4, exps_t)):
                    lg = wpool.tile([128, G, 4], F32, tag="lg", name=f"lg{nm}")
                    nc.vector.tensor_add(lg[:, :gn, :],
                                         src[:, :gn, alo:alo + 4],
                                         dstr[:, :gn, dlo:dlo + 4])
                    exp_lrelu(es[:, t0:t0 + gn, :], lg[:, :gn, :], gn)
                wfm = wpool.tile([128, G, 260], F32, tag="wf", name="wfm")
                wfs = wpool.tile([128, G, 260], F32, tag="wf", name="wfs")
                for h in range(4):
                    nc.vector.tensor_mul(
                        wfm[:, :gn, 65 * h:65 * h + 65],
                        src[:, :gn, 65 * h:65 * h + 65],
                        expm_t[:, t0:t0 + gn, h:h + 1].to_broadcast(
                            [128, gn, 65]))
                    nc.vector.tensor_mul(
                        wfs[:, :gn, 65 * h:65 * h + 65],
                        src[:, :gn, 260 + 65 * h:260 + 65 * h + 65],
                        exps_t[:, t0:t0 + gn, h:h + 1].to_broadcast(
                            [128, gn, 65]))
                oh = build_onehot(t0, gn)
                for ti in range(gn):
                    t = t0 + ti
                    b = blk_of[t]
                    if t in first_of:
                        psums_m[b] = ppool.tile([128, 260], F32, tag="agg",
                                                name="aggm")
                        psums_s[b] = ppool.tile([128, 260], F32, tag="agg",
                                                name="aggs")
                    st = (t == tile_off[b])
                    sp = (t == tile_off[b + 1] - 1)
                    nc.tensor.matmul(out=psums_m[b][:], lhsT=oh[:, ti, :],
                                     rhs=wfm[:, ti, :], start=st, stop=sp)
                    nc.tensor.matmul(out=psums_s[b][:], lhsT=oh[:, ti, :],
                                     rhs=wfs[:, ti, :], start=st, stop=sp)
                    if t in last_of:
                        b = last_of[t]
                        nb = nrows_blk[b]
                        rrow = rpool.tile([128, 64], F32, tag="rrow",
                                          name="rrow")
                        for (ps, z_dram, bias_t, co) in (
                                (psums_m[b], zm_out, bm_t, 0),
                                (psums_s[b], zs_out, bs_t, 4)):
                            rec = make_rec(ps, nb)
                            nc.scalar.copy(rrow[:nb, co:co + 4], rec[:nb, :4])
                            acc = flush_mean(ps, nb, rec, bias_t)
                            nc.sync.dma_start(z_dram[b * 128:b * 128 + nb, :],
                                              acc[:nb, :])
                        nc.sync.dma_start(Rms[b * 128:b * 128 + nb, :],
                                          rrow[:nb, :])
                        del psums_m[b], psums_s[b]
            for g in range(NGRP):
                t0 = g * G
                gn = min(G, T - t0)
                rt = gpool.tile([128, G, 64], F32, tag="dst", name="rt")
                gather(rt[:, :gn, :], Rms.ap(), dloc_t[:, t0 * 8:(t0 + gn) * 8],
                       gn * 128, 64, 3)
                atm = wpool.tile([128, G, 4], F32, tag="at", name="atm")
                nc.vector.tensor_mul(atm[:, :gn, :], expm_t[:, t0:t0 + gn, :],
                                     rt[:, :gn, 0:4])
                nc.sync.dma_start(a_outs["am"][:, t0:t0 + gn, :],
                                  atm[:, :gn, :])
                ats = wpool.tile([128, G, 4], F32, tag="at", name="ats")
                nc.vector.tensor_mul(ats[:, :gn, :], exps_t[:, t0:t0 + gn, :],
                                     rt[:, :gn, 4:8])
                nc.sync.dma_start(a_outs["as_"][:, t0:t0 + gn, :],
                                  ats[:, :gn, :])

        # ================= pipeline =================
        for t in range(NBLK):
            nt = nrows_blk[t]
            o = t * 128
            ps = dpool.tile([128, 64], F32, tag="dps", name="dps0")
            nc.tensor.matmul(out=ps[:nt, :], lhsT=xT_t[:, o:o + nt],
                             rhs=rhs0_t[:, :], start=True, stop=True)
            row = rpool.tile([128, 64], F32, tag="row", name="row0")
            nc.scalar.copy(row[:nt, :], ps[:nt, :])
            nc.sync.dma_start(T0_loc[o:o + nt, :], row[:nt, :])
        allgather(T0_loc, T0)
        h0T_t = sparse_gcn()

        exp1_t = epool.tile([128, T, 4], F32, tag="exp", name="exp1")
        dense_phase([(h0T_t, 64)], [rhs1_t], T1_loc, 320, D1)
        allgather(T1_loc, T1)
        h1T = sparse_gat(T1, D1, R1, 320, exp1_t, b1_t, True, 2, a_outs["a1"])

        exp2_t = epool.tile([128, T, 4], F32, tag="exp", name="exp2")
        dense_phase([(h1T[0], 128), (h1T[1], 128)],
                    [rhs2a_t, rhs2b_t], T2_loc, 320, D2)
        allgather(T2_loc, T2)
        h2T = sparse_gat(T2, D2, R2, 320, exp2_t, b2_t, False, 1, a_outs["a2"])
        h2T_t = h2T[0]

        expm_t = epool.tile([128, T, 4], F32, tag="exp", name="expm")
        exps_t = epool.tile([128, T, 4], F32, tag="exp", name="exps")
        dense_phase_ms()
        allgather(Tms_loc, Tms)
        sparse_ms()

    nc.compile()
    return nc


# ---------------------------------------------------------------- driver ---

_CACHE = {}


def kernel(**inputs):
    x = np.asarray(inputs["x"], np.float32)
    edge_index = np.asarray(inputs["edge_index"])
    per_core, tpb, tile_off, T, perm = _prep(edge_index.astype(np.int64))

    W = {k: np.asarray(v, np.float32) for k, v in inputs.items()
         if k not in ("x", "edge_index")}

    def rhs_pack(Wm, asrc, adst):
        # [ilv(260) | als 4 | ald 4]
        return np.concatenate(
            [_interleave_cols(Wm), Wm @ _blockdiag(asrc),
             Wm @ _blockdiag(adst)], axis=1).astype(np.float32)

    key = ("prog", T, tuple(tpb))
    if key not in _CACHE:
        _CACHE[key] = _build(tpb, T)
    nc = _CACHE[key]

    iota = np.broadcast_to(np.arange(128, dtype=np.float32),
                           (128, 1, 128)).copy()
    ident = np.eye(128, dtype=np.float32)
    rhs2 = rhs_pack(W["gat2_W"], W["gat2_asrc"], W["gat2_adst"])
    consts = dict(
        iota=iota, ident=ident,
        rhs0=W["gcn_W"].astype(np.float32),
        rhs1=rhs_pack(W["gat1_W"], W["gat1_asrc"], W["gat1_adst"]),
        rhs2a=np.ascontiguousarray(rhs2[:128]),
        rhs2b=np.ascontiguousarray(rhs2[128:]),
        rhsm=rhs_pack(W["mean_W"], W["mean_asrc"], W["mean_adst"]),
        rhss=rhs_pack(W["std_W"], W["std_asrc"], W["std_adst"]),
        b0b=np.broadcast_to(W["gcn_b"], (128, 64)).copy(),
        b1b=np.broadcast_to(W["gat1_b"], (128, 256)).copy(),
        b2b=np.broadcast_to(W["gat2_b"], (128, 64)).copy(),
        bmb=np.broadcast_to(W["mean_b"], (128, 64)).copy(),
        bsb=np.broadcast_to(W["std_b"], (128, 64)).copy(),
    )
    in_maps = []
    for k in range(NCORES):
        pc = per_core[k]
        xk = x[k * VSH:(k + 1) * VSH, :]
        in_maps.append(dict(
            xT=np.ascontiguousarray(xk.T),
            sidx=pc["sidx_w"], dloc=pc["dloc_w"],
            dblk=np.ascontiguousarray(pc["dblk_part"].astype(np.float32)),
            norm=np.ascontiguousarray(pc["norm_part"].astype(np.float32)),
            **consts))

    res = run_bass_kernel_spmd(nc, in_maps, core_ids=list(range(NCORES)))

    zm = np.concatenate([res.results[k]["zm"] for k in range(NCORES)], axis=0)
    zs = np.concatenate([res.results[k]["zs"] for k in range(NCORES)], axis=0)

    def alpha_full(nm):
        out = np.zeros((EP, 4), np.float32)
        for k in range(NCORES):
            a = res.results[k][nm]
            flat = np.ascontiguousarray(a.transpose(1, 0, 2)).reshape(-1, 4)
            spos = per_core[k]["spos"]
            m = spos >= 0
            out[spos[m]] = flat[m]
        inv = np.empty(EP, np.int64)
        inv[perm] = np.arange(EP)
        return out[inv]

    return (zm, zs, alpha_full("a1"), alpha_full("a2"),
            alpha_full("am"), alpha_full("as_"))
